# revision 6
# baseline (speedup 1.0000x reference)
"""Trainium2 Bass kernel for the stacked-attention module (8 NeuronCores).

Pure data parallel over batch (B=128 -> 16 batches/core, processed as 8
pairs with the pair side-by-side in the matmul free dim).

v4 (from v3 @263us): attack the ScalarE bottleneck (v3: 357 ACTIVATEs,
~215us busy, 87% occupancy; ~105us of that is the ~352c/instr fixed
overhead):
  * Hop bias (v_q_t) is injected INTO PSUM by a cheap 17-row fp16
    matmul per (pair, kt) bank: lhsT = vqtT[17, 128] (rows 0..15 =
    vqt[k, b]*256 per batch, row 16 = b_u*256), rhs = D17[17, 2, 196]
    one-hot batch-indicator rows + all-ones row 16.  This makes the
    PSUM already contain (v_i_t + v_q_t)*256, so tanh needs no ACT
    bias and can merge across batches AND kt tiles.
  * PSUM restructured into 2 tiles of 4 banks [128, 4, 512]; tanh is
    ONE ACT instruction per tile (FD=1568, PSUM strided read across
    banks), exp ONE per tile into em[:,0]; hop tanh instruction count
    drops 256 -> 32.
  * vqt is computed already-transposed: stationary = u (bf16), moving
    = wu -> PSUM [16, 512] x2, then one DVE copy *256 -> fp16 vqtT.
  * vi8 casts merged per half-pair (2 instrs instead of 8).
  * Schedule: l1(0), l1(1) first (PE density early), then h0(p-1)
    pipelined under l1(p); h1(3) bridges the h0(7)->vqtT1g2 boundary.

Softmax over the spatial dim needs no max subtraction (logits are tanh
outputs in (-1,1)) and p is never normalized: u += (sum e*vi) / (sum e).

Hop matmuls run in fp8(e4m3) with perf_mode=DoubleRow (w_vi scaled by 256
on host; compensated via the tanh activation's scale input). l1 likewise
fp8 DoubleRow (x*16, w*256, scale 1/4096).

Host-side (untimed) packing puts every tensor in exact SBUF layout:
  vi   [pair, xch, p, ctc, 392]  f8   (x * 16)
  vq   [p, ht, b, t]             bf16
  w1   [g, p, 2, m]              f8   (= l1_w.T tiles * 256)
  wvi* [p, ht, k]                f8   (= w_vi.T * 256)
  wu*  [p, ht, k]                bf16 (= w_u.T)
  l1b  [p, ht] f32
  vqtT*_init [17, k] fp16 (rows 0-15 zero, row 16 = b_u * 256)
  d17  [17, b, s] fp16 one-hot indicator
  out  [pair, p, kt, b]          f32  (u transposed; host untransposes)
"""

import numpy as np
from ml_dtypes import bfloat16, float8_e4m3
float16 = np.float16

import concourse.bass as bass
import concourse.tile as tile
from concourse import bacc, mybir
from concourse.bass import ts, ds
from concourse.bass_utils import run_bass_kernel_spmd

BF = mybir.dt.bfloat16
F8 = mybir.dt.float8e4
F16 = mybir.dt.float16
F32 = mybir.dt.float32

NCORES = 8
B = 128
C = 2048
S = 196
HID = 1024
T = 20
BL = B // NCORES
NPAIR = BL // 2
CT = C // 128
HT = HID // 128
S2 = 2 * S
NF8 = 16                   # all l1 ct-tiles contracted in fp8
NG8 = NF8 // 2             # DoubleRow groups for l1

WV_SCALE = 256.0

_NC = None


def _build():
    nc = bacc.Bacc(None)

    vi8x_p = nc.declare_dram_parameter("vi8x", [NPAIR, NG8, 128, 2, S2], F8, isOutput=False)
    vq_p = nc.declare_dram_parameter("vq", [128, HT, BL, T], BF, isOutput=False)
    w1f8_p = nc.declare_dram_parameter("w1f8", [NG8, 128, 2, HID], F8, isOutput=False)
    wvi0_p = nc.declare_dram_parameter("wvi0", [128, HT, HID], F8, isOutput=False)
    wu0_p = nc.declare_dram_parameter("wu0", [128, HT, HID], BF, isOutput=False)
    wvi1_p = nc.declare_dram_parameter("wvi1", [128, HT, HID], F8, isOutput=False)
    wu1_p = nc.declare_dram_parameter("wu1", [128, HT, HID], BF, isOutput=False)
    l1b_p = nc.declare_dram_parameter("l1b", [128, HT], F32, isOutput=False)
    vqt0i_p = nc.declare_dram_parameter("vqt0i", [17, HID], F16, isOutput=False)
    vqt1i_p = nc.declare_dram_parameter("vqt1i", [17, HID], F16, isOutput=False)
    d17_p = nc.declare_dram_parameter("d17", [17, BL, S], F16, isOutput=False)
    out_p = nc.declare_dram_parameter("out", [NPAIR, 128, HT, 2], F32, isOutput=True)

    Tanh = mybir.ActivationFunctionType.Tanh
    Exp = mybir.ActivationFunctionType.Exp
    X = mybir.AxisListType.X
    ADD = mybir.AluOpType.add
    MULT = mybir.AluOpType.mult

    with tile.TileContext(nc) as tc:
        with (
            tc.tile_pool(name="weights", bufs=1) as wpool,
            tc.tile_pool(name="xin", bufs=11) as xpool,
            tc.tile_pool(name="vis", bufs=1) as vipool,
            tc.tile_pool(name="small", bufs=1) as spool,
            tc.tile_pool(name="uu", bufs=3) as upool,
            tc.tile_pool(name="ha", bufs=3) as hapool,
            tc.tile_pool(name="em", bufs=3) as empool,
            tc.tile_pool(name="emf", bufs=3) as emfpool,
            tc.tile_pool(name="mm", bufs=2, space="PSUM") as mmpool,
        ):
            # ---- DMA staging.  gpsimd: w1f8 g0-3, wvi0, small consts.
            # scalar: vq, w1f8 g4-7, wu0b, wvi1, wu1b.  sync: x(pair),
            # wu0a, wu1a, outs. ----
            def emit_x(pair):
                x8c = []
                for i in range(NG8):
                    x8_sb = xpool.tile([128, 2, S2], F8, tag="x8", name=f"x8{pair}_{i}")
                    nc.sync.dma_start(out=x8_sb[:], in_=vi8x_p[pair, i])
                    x8c.append(x8_sb)
                return x8c

            xc0 = emit_x(0)

            w1f8_sb = []
            for g in range(NG8):
                w1f8c = wpool.tile([128, 2, HID], F8, tag=f"w1f8c{g}", name=f"w1f8c{g}")
                (nc.gpsimd if g < 4 else nc.scalar).dma_start(out=w1f8c[:], in_=w1f8_p[g])
                w1f8_sb.append(w1f8c)

            vq_sb = wpool.tile([128, HT, BL, T], BF, tag="vq")
            nc.scalar.dma_start(out=vq_sb[:], in_=vq_p[:])
            l1b_sb = wpool.tile([128, HT], F32, tag="l1b")
            nc.scalar.dma_start(out=l1b_sb[:], in_=l1b_p[:])

            d17_sb = wpool.tile([17, BL, S], F16, tag="d17")
            nc.gpsimd.dma_start(out=d17_sb[:], in_=d17_p[:])
            vqtT_sb = []
            for h, p_ in ((0, vqt0i_p), (1, vqt1i_p)):
                t_ = wpool.tile([17, HID], F16, tag=f"vqtT{h}", name=f"vqtT{h}")
                nc.gpsimd.dma_start(out=t_[:], in_=p_[:])
                vqtT_sb.append(t_)

            wvi_sb = []
            wu_sb = []
            for i, (wvi_p, wu_p) in enumerate(((wvi0_p, wu0_p), (wvi1_p, wu1_p))):
                wv = wpool.tile([128, HT, HID], F8, tag=f"wvi{i}", name=f"wvi{i}")
                (nc.gpsimd if i == 0 else nc.scalar).dma_start(out=wv[:], in_=wvi_p[:])
                wvi_sb.append(wv)
                wu = wpool.tile([128, HT, HID], BF, tag=f"wu{i}", name=f"wu{i}")
                nc.sync.dma_start(out=wu[:, : HT // 2], in_=wu_p[:, : HT // 2])
                (nc.scalar if i == 0 else nc.gpsimd).dma_start(
                    out=wu[:, HT // 2 :], in_=wu_p[:, HT // 2 :]
                )
                wu_sb.append(wu)

            # ---- u0 = mean_t(v_q) ----
            u_t = [upool.tile([128, HT, BL], F32, tag="u", name=f"u{h}") for h in range(3)]
            ubf_t = [spool.tile([128, HT, BL], BF, tag=f"ubf{h}", name=f"ubf{h}") for h in range(2)]
            u0 = u_t[0]
            for ht in range(HT):
                nc.vector.reduce_sum(out=u0[:, ht, :], in_=vq_sb[:, ht, :, :], axis=X)
            nc.vector.tensor_scalar_mul(out=u0[:], in0=u0[:], scalar1=1.0 / T)
            nc.vector.tensor_copy(out=ubf_t[0][:], in_=u0[:])

            vi_bf = vipool.tile([128, HT, NPAIR, S2], BF, tag="vi")
            vi8 = vipool.tile([128, HT, NPAIR, S2], F8, tag="vi8", name="vi8")

            z_sb = [spool.tile([128, HT, BL], F32, tag=f"z{h}", name=f"z{h}") for h in range(2)]
            r_sb = [spool.tile([128, HT, BL], F32, tag=f"r{h}", name=f"r{h}") for h in range(2)]
            zr_sb = [spool.tile([128, HT, BL], F32, tag=f"zr{h}", name=f"zr{h}") for h in range(2)]
            upd_sb = [spool.tile([128, HT, BL], F32, tag=f"upd{h}", name=f"upd{h}") for h in range(2)]

            # ---- vqtT emission: PE-transposed vqt, then *256 -> fp16 ----
            def emit_vqtT(hop, ubf, rows):
                """rows: slice of batches to produce (always computes from
                batch 0 up so output partitions align)."""
                hi = rows.stop
                vps = mmpool.tile([128, 4, 512], F32, tag="mm", name=f"vqtps{hop}_{hi}")
                for kh in range(2):
                    for ht in range(HT):
                        nc.tensor.matmul(
                            vps[:hi, kh, :],
                            ubf[:, ht, :hi],
                            wu_sb[hop][:, ht, ts(kh, 512)],
                            start=(ht == 0),
                            stop=(ht == HT - 1),
                        )
                # rows 0..hi computed and copied (PSUM reads must start at
                # partition 0; re-copying unchanged low rows is harmless)
                nc.vector.tensor_scalar(
                    out=vqtT_sb[hop][0:hi, :].rearrange("b (kh k) -> b kh k", kh=2),
                    in0=vps[0:hi, 0:2, :],
                    scalar1=WV_SCALE,
                    scalar2=None,
                    op0=MULT,
                )

            # ---- one hop half: 4 kt banks -> tanh -> exp -> mul -> fold
            # -> 2 reduces ----
            def emit_hop_half(hop, pair, half):
                h4 = slice(4 * half, 4 * half + 4)
                ps4 = mmpool.tile([128, 4, 512], F32, tag="mm", name=f"mm{hop}_{pair}_{half}")
                for q in range(4):
                    kt = 4 * half + q
                    nc.tensor.matmul(
                        ps4[:, q, 0:S2],
                        vqtT_sb[hop][:, ts(kt, 128)],
                        d17_sb[:, ts(pair, 2), :],
                        start=True,
                        stop=False,
                    )
                    for h2 in range(HT // 2):
                        nc.tensor.matmul(
                            ps4[:, q, 0:S2],
                            wvi_sb[hop][:, 2 * h2 : 2 * h2 + 2, ts(kt, 128)],
                            vi8[:, 2 * h2 : 2 * h2 + 2, pair, :],
                            perf_mode=mybir.MatmulPerfMode.DoubleRow,
                            start=False,
                            stop=(h2 == HT // 2 - 1),
                        )
                ha = hapool.tile([128, 4, S2], BF, tag="ha", name=f"ha{hop}_{pair}_{half}")
                nc.scalar.activation(
                    out=ha[:], in_=ps4[:, :, 0:S2], func=Tanh, scale=1.0 / WV_SCALE
                )
                em = empool.tile([128, 2, 4, S2], BF, tag="em", name=f"em{hop}_{pair}_{half}")
                nc.scalar.activation(out=em[:, 0], in_=ha[:], func=Exp)
                nc.vector.tensor_mul(
                    out=em[:, 1], in0=em[:, 0], in1=vi_bf[:, h4, pair, :]
                )
                emv = em[:].rearrange("p e k (j s) -> p (e k j) s", j=2)
                emf = emfpool.tile([128, 16, S // 2], BF, tag="emf", name=f"emf{hop}_{pair}_{half}")
                nc.vector.tensor_add(
                    out=emf[:], in0=emv[:, :, : S // 2], in1=emv[:, :, S // 2 :]
                )
                ef_z = emf[:, 0:8, :].rearrange("p (k j) s -> p k j s", k=4)
                ef_r = emf[:, 8:16, :].rearrange("p (k j) s -> p k j s", k=4)
                nc.vector.reduce_sum(
                    out=z_sb[hop][:, h4, ts(pair, 2)], in_=ef_z, axis=X
                )
                nc.vector.reduce_sum(
                    out=r_sb[hop][:, h4, ts(pair, 2)], in_=ef_r, axis=X
                )

            def emit_upair_chain(hop, pair):
                c = ts(pair, 2)
                u_prev, u_next = u_t[hop], u_t[hop + 1]
                nc.vector.reciprocal(out=zr_sb[hop][:, :, c], in_=z_sb[hop][:, :, c])
                nc.vector.tensor_mul(
                    out=upd_sb[hop][:, :, c], in0=r_sb[hop][:, :, c], in1=zr_sb[hop][:, :, c]
                )
                nc.vector.tensor_add(
                    out=u_next[:, :, c], in0=u_prev[:, :, c], in1=upd_sb[hop][:, :, c]
                )
                if hop == 0:
                    nc.vector.tensor_copy(out=ubf_t[1][:, :, c], in_=u_next[:, :, c])

            def emit_hop_pair(hop, pair):
                emit_hop_half(hop, pair, 0)
                emit_hop_half(hop, pair, 1)
                emit_upair_chain(hop, pair)
                if hop == 1:
                    nc.sync.dma_start(out=out_p[pair], in_=u_t[2][:, :, ts(pair, 2)])

            # ---- l1 emitter (one pair) ----
            def emit_l1_pair(pair):
                x8c = xc0 if pair == 0 else emit_x(pair)
                for hh in range(2):
                    ps4 = mmpool.tile([128, 4, 512], F32, tag="mm", name=f"l1ps{pair}_{hh}")
                    for q in range(4):
                        ht = 4 * hh + q
                        for g in range(NG8):
                            nc.tensor.matmul(
                                ps4[:, q, 0:S2],
                                w1f8_sb[g][:, :, ts(ht, 128)],
                                x8c[g][:],
                                perf_mode=mybir.MatmulPerfMode.DoubleRow,
                                start=(g == 0),
                                stop=(g == NG8 - 1),
                            )
                        nc.scalar.activation(
                            out=vi_bf[:, ht, pair, :],
                            in_=ps4[:, q, 0:S2],
                            func=Tanh,
                            bias=l1b_sb[:, ht : ht + 1],
                            scale=1.0 / 4096.0,
                        )
                    nc.vector.tensor_copy(
                        out=vi8[:, ts(hh, 4), pair, :], in_=vi_bf[:, ts(hh, 4), pair, :]
                    )

            # ---- schedule ----
            # P1: front-load two l1 pairs for PE density, then pipeline
            # h0(p-1) under l1(p).
            emit_l1_pair(0)
            emit_l1_pair(1)
            emit_vqtT(0, ubf_t[0], slice(0, BL))
            emit_hop_pair(0, 0)
            for pair in range(2, NPAIR):
                emit_l1_pair(pair)
                emit_hop_pair(0, pair - 1)
                if pair == 5:
                    # u1 for pairs 0-3 complete after h0(3) (emitted
                    # under l1(4)); vqtT1 group 1 -> batches 0-7
                    emit_vqtT(1, ubf_t[1], slice(0, 8))
                if pair >= 5:
                    emit_hop_pair(1, pair - 5)
            emit_hop_pair(0, NPAIR - 1)
            emit_hop_pair(1, 3)
            emit_vqtT(1, ubf_t[1], slice(8, 16))
            for pair in range(4, NPAIR):
                emit_hop_pair(1, pair)

    nc.compile()
    return nc


def _get_nc():
    global _NC
    if _NC is None:
        _NC = _build()
    return _NC


def _prep_in_maps(v_i, v_q, l1_w, l1_b, w_vi0, w_u0, b_u0, w_vi1, w_u1, b_u1):
    v_i = np.asarray(v_i, np.float32)
    v_q = np.asarray(v_q, np.float32)

    # vi: [B, C, H, W] -> [core, pair, p, ct, j, s]; all ct fp8*16
    vif = v_i.reshape(NCORES, NPAIR, 2, CT, 128, S).transpose(0, 1, 4, 3, 2, 5)
    vif = np.ascontiguousarray(vif)  # [core, pair, p, ct, j, s] f32
    vi8x = (vif * 16.0).astype(float8_e4m3).reshape(
        NCORES, NPAIR, 128, NG8, 2, S2
    )
    vi8x = np.ascontiguousarray(vi8x.transpose(0, 1, 3, 2, 4, 5))

    # vq: [B, T, HID] -> [core, p, ht, b, t]
    vq = v_q.reshape(NCORES, BL, T, HT, 128).transpose(0, 4, 3, 1, 2)
    vq = np.ascontiguousarray(vq.astype(bfloat16))

    def packT(w, ntiles, dt, scale=1.0):
        wt = (np.asarray(w, np.float32).T * scale).astype(dt)
        return np.ascontiguousarray(
            wt.reshape(ntiles, 128, w.shape[0]).transpose(1, 0, 2)
        )

    # w1: all ct fp8*256 as [g, p, 2, m]
    w1t = np.asarray(l1_w, np.float32).T.reshape(CT, 128, HID)  # [ct, p, m]
    w1f8h = np.ascontiguousarray(
        (w1t * 256.0)
        .astype(float8_e4m3)
        .reshape(NG8, 2, 128, HID)
        .transpose(0, 2, 1, 3)
    )

    wvi0h = packT(w_vi0, HT, float8_e4m3, WV_SCALE)
    wvi1h = packT(w_vi1, HT, float8_e4m3, WV_SCALE)
    wu0h = packT(w_u0, HT, bfloat16)
    wu1h = packT(w_u1, HT, bfloat16)

    l1bh = np.ascontiguousarray(np.asarray(l1_b, np.float32).reshape(HT, 128).T)

    def vqt_init(b_u):
        v = np.zeros((17, HID), np.float32)
        v[16] = np.asarray(b_u, np.float32) * WV_SCALE
        return v.astype(float16)

    vqt0i = vqt_init(b_u0)
    vqt1i = vqt_init(b_u1)

    d17 = np.zeros((17, BL, S), np.float32)
    for b in range(BL):
        d17[b, b, :] = 1.0
    d17[16, :, :] = 1.0
    d17 = d17.astype(float16)

    in_maps = []
    for core in range(NCORES):
        in_maps.append(
            {
                "vi8x": vi8x[core],
                "vq": vq[core],
                "w1f8": w1f8h,
                "wvi0": wvi0h,
                "wu0": wu0h,
                "wvi1": wvi1h,
                "wu1": wu1h,
                "l1b": l1bh,
                "vqt0i": vqt0i,
                "vqt1i": vqt1i,
                "d17": d17,
            }
        )
    return in_maps


def run_sharded(inputs: dict, trace: bool = False):
    """Returns (full_output [128,1024] f32, BassKernelResults)."""
    nc = _get_nc()
    in_maps = _prep_in_maps(**inputs)
    res = run_bass_kernel_spmd(
        nc, in_maps, core_ids=list(range(NCORES)), trace=trace
    )
    outs = []
    for i in range(NCORES):
        o = np.asarray(res.results[i]["out"])  # [pair, p, kt, j]
        outs.append(
            np.ascontiguousarray(o.transpose(0, 3, 2, 1)).reshape(BL, HID)
        )
    full = np.concatenate(outs, axis=0).astype(np.float32)
    return full, res


def kernel(**inputs) -> np.ndarray:
    out, _ = run_sharded(inputs, trace=False)
    return out


# revision 12
# speedup vs baseline: 1.0925x; 1.0925x over previous
"""Trainium2 Bass kernel for the stacked-attention module (8 NeuronCores).

Pure data parallel over batch (B=128 -> 16 batches/core, processed as 8
pairs with the pair side-by-side in the matmul free dim).

v4 (from v3 @263us): attack the ScalarE bottleneck (v3: 357 ACTIVATEs,
~215us busy, 87% occupancy; ~105us of that is the ~352c/instr fixed
overhead):
  * Hop bias (v_q_t) is injected INTO PSUM by a cheap 17-row fp16
    matmul per (pair, kt) bank: lhsT = vqtT[17, 128] (rows 0..15 =
    vqt[k, b]*256 per batch, row 16 = b_u*256), rhs = D17[17, 2, 196]
    one-hot batch-indicator rows + all-ones row 16.  This makes the
    PSUM already contain (v_i_t + v_q_t)*256, so tanh needs no ACT
    bias and can merge across batches AND kt tiles.
  * PSUM restructured into 2 tiles of 4 banks [128, 4, 512]; tanh is
    ONE ACT instruction per tile (FD=1568, PSUM strided read across
    banks), exp ONE per tile into em[:,0]; hop tanh instruction count
    drops 256 -> 32.
  * vqt is computed already-transposed: stationary = u (bf16), moving
    = wu -> PSUM [16, 512] x2, then one DVE copy *256 -> fp16 vqtT.
  * vi8 casts merged per half-pair (2 instrs instead of 8).
  * Schedule: l1(0), l1(1) first (PE density early), then h0(p-1)
    pipelined under l1(p); h1(3) bridges the h0(7)->vqtT1g2 boundary.

Softmax over the spatial dim needs no max subtraction (logits are tanh
outputs in (-1,1)) and p is never normalized: u += (sum e*vi) / (sum e).

Hop matmuls run in fp8(e4m3) with perf_mode=DoubleRow (w_vi scaled by 256
on host; compensated via the tanh activation's scale input). l1 likewise
fp8 DoubleRow (x*16, w*256, scale 1/4096).

Host-side (untimed) packing puts every tensor in exact SBUF layout:
  vi   [pair, xch, p, ctc, 392]  f8   (x * 16)
  vq   [p, ht, b, t]             bf16
  w1   [g, p, 2, m]              f8   (= l1_w.T tiles * 256)
  wvi* [p, ht, k]                f8   (= w_vi.T * 256)
  wu*  [p, ht, k]                bf16 (= w_u.T)
  l1b  [p, ht] f32
  vqtT*_init [17, k] fp16 (rows 0-15 zero, row 16 = b_u * 256)
  d17  [17, b, s] fp16 one-hot indicator
  out  [pair, p, kt, b]          f32  (u transposed; host untransposes)
"""

import numpy as np
from ml_dtypes import bfloat16, float8_e4m3
float16 = np.float16

import concourse.bass as bass
import concourse.tile as tile
from concourse import bacc, mybir
from concourse.bass import ts, ds
from concourse.bass_utils import run_bass_kernel_spmd

BF = mybir.dt.bfloat16
F8 = mybir.dt.float8e4
F16 = mybir.dt.float16
F32 = mybir.dt.float32

NCORES = 8
B = 128
C = 2048
S = 196
HID = 1024
T = 20
BL = B // NCORES
NPAIR = BL // 2
CT = C // 128
HT = HID // 128
S2 = 2 * S
NF8 = 16                   # all l1 ct-tiles contracted in fp8
NG8 = NF8 // 2             # DoubleRow groups for l1

WV_SCALE = 256.0

_NC = None


def _build():
    nc = bacc.Bacc(None)

    vi8x_p = nc.declare_dram_parameter("vi8x", [NPAIR, NG8, 128, 2, S2], F8, isOutput=False)
    vq_p = nc.declare_dram_parameter("vq", [128, HT, BL, T], BF, isOutput=False)
    w1f8_p = nc.declare_dram_parameter("w1f8", [NG8, 128, 2, HID], F8, isOutput=False)
    wvi0_p = nc.declare_dram_parameter("wvi0", [128, HT, HID], F8, isOutput=False)
    wu0_p = nc.declare_dram_parameter("wu0", [128, HT, HID], BF, isOutput=False)
    wvi1_p = nc.declare_dram_parameter("wvi1", [128, HT, HID], F8, isOutput=False)
    wu1_p = nc.declare_dram_parameter("wu1", [128, HT, HID], BF, isOutput=False)
    l1b_p = nc.declare_dram_parameter("l1b", [128, HT], F32, isOutput=False)
    vqt0i_p = nc.declare_dram_parameter("vqt0i", [128, HID], F16, isOutput=False)
    vqt1i_p = nc.declare_dram_parameter("vqt1i", [128, HID], F16, isOutput=False)
    d17_p = nc.declare_dram_parameter("d17", [128, BL, S], F16, isOutput=False)
    out_p = nc.declare_dram_parameter("out", [NPAIR, 128, HT, 2], F32, isOutput=True)

    Tanh = mybir.ActivationFunctionType.Tanh
    Exp = mybir.ActivationFunctionType.Exp
    X = mybir.AxisListType.X
    ADD = mybir.AluOpType.add
    MULT = mybir.AluOpType.mult

    with tile.TileContext(nc) as tc:
        with (
            tc.tile_pool(name="weights", bufs=1) as wpool,
            tc.tile_pool(name="xin", bufs=11) as xpool,
            tc.tile_pool(name="vis", bufs=1) as vipool,
            tc.tile_pool(name="small", bufs=1) as spool,
            tc.tile_pool(name="uu", bufs=3) as upool,
            tc.tile_pool(name="ha", bufs=3) as hapool,
            tc.tile_pool(name="em", bufs=3) as empool,
            tc.tile_pool(name="emf", bufs=3) as emfpool,
            tc.tile_pool(name="mm", bufs=2, space="PSUM") as mmpool,
        ):
            # ---- DMA staging.  sync queue carries ONLY the x chunks (and
            # tiny outs) so pair p+1's x is never stuck behind weights.
            # gpsimd: w1f8 g0-3, consts, wu0a, wvi0, wu1b.
            # scalar: w1f8 g4-7, vq, l1b, wu0b, wvi1, wu1a. ----
            xq = {}

            def emit_x(pair):
                x8c = []
                for i in range(NG8):
                    x8_sb = xpool.tile([128, 2, S2], F8, tag="x8", name=f"x8{pair}_{i}")
                    nc.sync.dma_start(out=x8_sb[:], in_=vi8x_p[pair, i])
                    x8c.append(x8_sb)
                xq[pair] = x8c

            emit_x(0)
            emit_x(1)

            w1f8_sb = []
            for g in range(NG8):
                w1f8c = wpool.tile([128, 2, HID], F8, tag=f"w1f8c{g}", name=f"w1f8c{g}")
                (nc.gpsimd if g % 2 == 0 else nc.scalar).dma_start(out=w1f8c[:], in_=w1f8_p[g])
                w1f8_sb.append(w1f8c)

            d17_sb = wpool.tile([128, BL, S], F16, tag="d17")
            nc.gpsimd.dma_start(out=d17_sb[:], in_=d17_p[:])
            vqtT_sb = []
            for h, p_ in ((0, vqt0i_p), (1, vqt1i_p)):
                t_ = wpool.tile([128, HID], F16, tag=f"vqtT{h}", name=f"vqtT{h}")
                nc.gpsimd.dma_start(out=t_[:], in_=p_[:])
                vqtT_sb.append(t_)

            vq_sb = wpool.tile([128, HT, BL, T], BF, tag="vq")
            nc.scalar.dma_start(out=vq_sb[:], in_=vq_p[:])
            l1b_sb = wpool.tile([128, HT], F32, tag="l1b")
            nc.scalar.dma_start(out=l1b_sb[:], in_=l1b_p[:])

            wu_sb = []
            wvi_sb = []
            for i, (wvi_p, wu_p) in enumerate(((wvi0_p, wu0_p), (wvi1_p, wu1_p))):
                wu = wpool.tile([128, HT, HID], BF, tag=f"wu{i}", name=f"wu{i}")
                (nc.gpsimd if i == 0 else nc.scalar).dma_start(
                    out=wu[:, : HT // 2], in_=wu_p[:, : HT // 2]
                )
                (nc.scalar if i == 0 else nc.gpsimd).dma_start(
                    out=wu[:, HT // 2 :], in_=wu_p[:, HT // 2 :]
                )
                wu_sb.append(wu)
                wv = wpool.tile([128, HT, HID], F8, tag=f"wvi{i}", name=f"wvi{i}")
                (nc.gpsimd if i == 0 else nc.scalar).dma_start(out=wv[:], in_=wvi_p[:])
                wvi_sb.append(wv)

            # ---- u0 = mean_t(v_q) ----
            u_t = [upool.tile([128, HT, BL], F32, tag="u", name=f"u{h}") for h in range(3)]
            ubf_t = [spool.tile([128, HT, BL], BF, tag=f"ubf{h}", name=f"ubf{h}") for h in range(2)]
            u0 = u_t[0]
            for ht in range(HT):
                nc.vector.reduce_sum(out=u0[:, ht, :], in_=vq_sb[:, ht, :, :], axis=X)
            nc.vector.tensor_scalar_mul(out=u0[:], in0=u0[:], scalar1=1.0 / T)
            nc.vector.tensor_copy(out=ubf_t[0][:], in_=u0[:])

            vi_bf = vipool.tile([128, HT, NPAIR, S2], BF, tag="vi")
            vi8 = vipool.tile([128, HT, NPAIR, S2], F8, tag="vi8", name="vi8")

            z_sb = [spool.tile([128, HT, BL], F32, tag=f"z{h}", name=f"z{h}") for h in range(2)]
            r_sb = [spool.tile([128, HT, BL], F32, tag=f"r{h}", name=f"r{h}") for h in range(2)]
            zr_sb = [spool.tile([128, HT, BL], F32, tag=f"zr{h}", name=f"zr{h}") for h in range(2)]
            upd_sb = [spool.tile([128, HT, BL], F32, tag=f"upd{h}", name=f"upd{h}") for h in range(2)]

            # ---- vqtT emission: PE-transposed vqt, then *256 -> fp16 ----
            def emit_vqtT(hop, ubf, rows):
                """rows: slice of batches to produce (always computes from
                batch 0 up so output partitions align)."""
                hi = rows.stop
                vps = mmpool.tile([128, 4, 512], F32, tag="mm", name=f"vqtps{hop}_{hi}")
                for kh in range(2):
                    for ht in range(HT):
                        nc.tensor.matmul(
                            vps[:hi, kh, :],
                            ubf[:, ht, :hi],
                            wu_sb[hop][:, ht, ts(kh, 512)],
                            start=(ht == 0),
                            stop=(ht == HT - 1),
                        )
                # rows 0..hi computed and copied (PSUM reads must start at
                # partition 0; re-copying unchanged low rows is harmless)
                nc.vector.tensor_scalar(
                    out=vqtT_sb[hop][0:hi, :].rearrange("b (kh k) -> b kh k", kh=2),
                    in0=vps[0:hi, 0:2, :],
                    scalar1=WV_SCALE,
                    scalar2=None,
                    op0=MULT,
                )

            # ---- one hop half: 4 kt banks -> tanh -> exp -> mul -> fold
            # -> 2 reduces ----
            def emit_hop_half(hop, pair, half):
                h4 = slice(4 * half, 4 * half + 4)
                ps4 = mmpool.tile([128, 4, 512], F32, tag="mm", name=f"mm{hop}_{pair}_{half}")
                # all 4 bias matmuls first (one stationary row-config), then
                # the fp8 DoubleRow stacks
                for q in range(4):
                    kt = 4 * half + q
                    nc.tensor.matmul(
                        ps4[:, q, 0:S2],
                        vqtT_sb[hop][:, ts(kt, 128)],
                        d17_sb[:, ts(pair, 2), :],
                        start=True,
                        stop=False,
                    )
                for q in range(4):
                    kt = 4 * half + q
                    for h2 in range(HT // 2):
                        nc.tensor.matmul(
                            ps4[:, q, 0:S2],
                            wvi_sb[hop][:, 2 * h2 : 2 * h2 + 2, ts(kt, 128)],
                            vi8[:, 2 * h2 : 2 * h2 + 2, pair, :],
                            perf_mode=mybir.MatmulPerfMode.DoubleRow,
                            start=False,
                            stop=(h2 == HT // 2 - 1),
                        )
                ha = hapool.tile([128, 4, S2], BF, tag="ha", name=f"ha{hop}_{pair}_{half}")
                nc.scalar.activation(
                    out=ha[:], in_=ps4[:, :, 0:S2], func=Tanh, scale=1.0 / WV_SCALE
                )
                em = empool.tile([128, 2, 4, S2], BF, tag="em", name=f"em{hop}_{pair}_{half}")
                nc.scalar.activation(out=em[:, 0], in_=ha[:], func=Exp)
                nc.vector.tensor_mul(
                    out=em[:, 1], in0=em[:, 0], in1=vi_bf[:, h4, pair, :]
                )
                emv = em[:].rearrange("p e k (j s) -> p (e k j) s", j=2)
                emf = emfpool.tile([128, 16, S // 2], BF, tag="emf", name=f"emf{hop}_{pair}_{half}")
                nc.vector.tensor_add(
                    out=emf[:], in0=emv[:, :, : S // 2], in1=emv[:, :, S // 2 :]
                )
                ef_z = emf[:, 0:8, :].rearrange("p (k j) s -> p k j s", k=4)
                ef_r = emf[:, 8:16, :].rearrange("p (k j) s -> p k j s", k=4)
                nc.vector.reduce_sum(
                    out=z_sb[hop][:, h4, ts(pair, 2)], in_=ef_z, axis=X
                )
                nc.vector.reduce_sum(
                    out=r_sb[hop][:, h4, ts(pair, 2)], in_=ef_r, axis=X
                )

            def emit_upair_chain(hop, pair):
                c = ts(pair, 2)
                u_prev, u_next = u_t[hop], u_t[hop + 1]
                nc.vector.reciprocal(out=zr_sb[hop][:, :, c], in_=z_sb[hop][:, :, c])
                nc.vector.tensor_mul(
                    out=upd_sb[hop][:, :, c], in0=r_sb[hop][:, :, c], in1=zr_sb[hop][:, :, c]
                )
                nc.vector.tensor_add(
                    out=u_next[:, :, c], in0=u_prev[:, :, c], in1=upd_sb[hop][:, :, c]
                )
                if hop == 0:
                    nc.vector.tensor_copy(out=ubf_t[1][:, :, c], in_=u_next[:, :, c])

            def emit_hop_pair(hop, pair):
                emit_hop_half(hop, pair, 0)
                emit_hop_half(hop, pair, 1)
                emit_upair_chain(hop, pair)
                if hop == 1:
                    nc.sync.dma_start(out=out_p[pair], in_=u_t[2][:, :, ts(pair, 2)])

            # ---- l1 emitter (one pair).  chunk_major orders the first
            # pair's matmuls g-major so the PE tracks the w1f8/x chunk DMAs
            # instead of head-of-line blocking on chunk g+1. ----
            def emit_l1_pair(pair, chunk_major=False):
                if pair + 1 < NPAIR and pair + 1 not in xq:
                    emit_x(pair + 1)  # prefetch next pair's x on sync queue
                x8c = xq.pop(pair)
                for hh in range(2):
                    ps4 = mmpool.tile([128, 4, 512], F32, tag="mm", name=f"l1ps{pair}_{hh}")
                    order = (
                        [(g, q) for g in range(NG8) for q in range(4)]
                        if chunk_major
                        else [(g, q) for q in range(4) for g in range(NG8)]
                    )
                    for g, q in order:
                        nc.tensor.matmul(
                            ps4[:, q, 0:S2],
                            w1f8_sb[g][:, :, ts(4 * hh + q, 128)],
                            x8c[g][:],
                            perf_mode=mybir.MatmulPerfMode.DoubleRow,
                            start=(g == 0),
                            stop=(g == NG8 - 1),
                        )
                    for q in range(4):
                        ht = 4 * hh + q
                        nc.scalar.activation(
                            out=vi_bf[:, ht, pair, :],
                            in_=ps4[:, q, 0:S2],
                            func=Tanh,
                            bias=l1b_sb[:, ht : ht + 1],
                            scale=1.0 / 4096.0,
                        )
                    nc.vector.tensor_copy(
                        out=vi8[:, ts(hh, 4), pair, :], in_=vi_bf[:, ts(hh, 4), pair, :]
                    )

            # ---- schedule ----
            # P1: front-load two l1 pairs for PE density, then pipeline
            # h0(p-1) and h1(p-4) under l1(p); vqtT1 is emitted in 2-pair
            # slices as soon as the needed u1 chains land.
            emit_l1_pair(0, chunk_major=True)
            emit_l1_pair(1)
            emit_vqtT(0, ubf_t[0], slice(0, BL))
            emit_hop_pair(0, 0)
            for pair in range(2, NPAIR):
                emit_l1_pair(pair)
                emit_hop_pair(0, pair - 1)
                if pair == 4:
                    emit_vqtT(1, ubf_t[1], slice(0, 4))   # u1(0,1) ready
                elif pair == 5:
                    emit_vqtT(1, ubf_t[1], slice(0, 8))   # u1(2,3) ready
                elif pair == 7:
                    emit_vqtT(1, ubf_t[1], slice(0, 12))  # u1(4,5) ready
                if pair >= 4:
                    emit_hop_pair(1, pair - 4)
            emit_hop_pair(0, NPAIR - 1)
            emit_hop_pair(1, 4)
            emit_hop_pair(1, 5)
            emit_vqtT(1, ubf_t[1], slice(0, BL))          # u1(6,7) ready
            emit_hop_pair(1, 6)
            emit_hop_pair(1, 7)

    nc.compile()
    return nc


def _get_nc():
    global _NC
    if _NC is None:
        _NC = _build()
    return _NC


def _prep_in_maps(v_i, v_q, l1_w, l1_b, w_vi0, w_u0, b_u0, w_vi1, w_u1, b_u1):
    v_i = np.asarray(v_i, np.float32)
    v_q = np.asarray(v_q, np.float32)

    # vi: [B, C, H, W] -> [core, pair, p, ct, j, s]; all ct fp8*16
    vif = v_i.reshape(NCORES, NPAIR, 2, CT, 128, S).transpose(0, 1, 4, 3, 2, 5)
    vif = np.ascontiguousarray(vif)  # [core, pair, p, ct, j, s] f32
    vi8x = (vif * 16.0).astype(float8_e4m3).reshape(
        NCORES, NPAIR, 128, NG8, 2, S2
    )
    vi8x = np.ascontiguousarray(vi8x.transpose(0, 1, 3, 2, 4, 5))

    # vq: [B, T, HID] -> [core, p, ht, b, t]
    vq = v_q.reshape(NCORES, BL, T, HT, 128).transpose(0, 4, 3, 1, 2)
    vq = np.ascontiguousarray(vq.astype(bfloat16))

    def packT(w, ntiles, dt, scale=1.0):
        wt = (np.asarray(w, np.float32).T * scale).astype(dt)
        return np.ascontiguousarray(
            wt.reshape(ntiles, 128, w.shape[0]).transpose(1, 0, 2)
        )

    # w1: all ct fp8*256 as [g, p, 2, m]
    w1t = np.asarray(l1_w, np.float32).T.reshape(CT, 128, HID)  # [ct, p, m]
    w1f8h = np.ascontiguousarray(
        (w1t * 256.0)
        .astype(float8_e4m3)
        .reshape(NG8, 2, 128, HID)
        .transpose(0, 2, 1, 3)
    )

    wvi0h = packT(w_vi0, HT, float8_e4m3, WV_SCALE)
    wvi1h = packT(w_vi1, HT, float8_e4m3, WV_SCALE)
    wu0h = packT(w_u0, HT, bfloat16)
    wu1h = packT(w_u1, HT, bfloat16)

    l1bh = np.ascontiguousarray(np.asarray(l1_b, np.float32).reshape(HT, 128).T)

    def vqt_init(b_u):
        v = np.zeros((128, HID), np.float32)
        v[16] = np.asarray(b_u, np.float32) * WV_SCALE
        return v.astype(float16)

    vqt0i = vqt_init(b_u0)
    vqt1i = vqt_init(b_u1)

    d17 = np.zeros((128, BL, S), np.float32)
    for b in range(BL):
        d17[b, b, :] = 1.0
    d17[16, :, :] = 1.0
    d17 = d17.astype(float16)

    in_maps = []
    for core in range(NCORES):
        in_maps.append(
            {
                "vi8x": vi8x[core],
                "vq": vq[core],
                "w1f8": w1f8h,
                "wvi0": wvi0h,
                "wu0": wu0h,
                "wvi1": wvi1h,
                "wu1": wu1h,
                "l1b": l1bh,
                "vqt0i": vqt0i,
                "vqt1i": vqt1i,
                "d17": d17,
            }
        )
    return in_maps


def run_sharded(inputs: dict, trace: bool = False):
    """Returns (full_output [128,1024] f32, BassKernelResults)."""
    nc = _get_nc()
    in_maps = _prep_in_maps(**inputs)
    res = run_bass_kernel_spmd(
        nc, in_maps, core_ids=list(range(NCORES)), trace=trace
    )
    outs = []
    for i in range(NCORES):
        o = np.asarray(res.results[i]["out"])  # [pair, p, kt, j]
        outs.append(
            np.ascontiguousarray(o.transpose(0, 3, 2, 1)).reshape(BL, HID)
        )
    full = np.concatenate(outs, axis=0).astype(np.float32)
    return full, res


def kernel(**inputs) -> np.ndarray:
    out, _ = run_sharded(inputs, trace=False)
    return out
